# revision 8
# baseline (speedup 1.0000x reference)
"""Trainium2 Bass kernel for MixerDiffAttention (differential attention).

Sharding: tensor-parallel over the 8 (n_head//2) head groups across 8 cores.
Each core computes QKV projections for its head group, both differential
attention branches, the normalized combination y1 - lambda*y2, then an
AllGather of per-head outputs and a column-shard of the final projection.

Math layout notes (per core, head h):
  - x is host-transposed to xT [D, T] so D (contraction dim) sits on SBUF
    partitions for all projection matmuls.
  - q/k are produced in [t, c] layout (rmsnorm reduce + rotary are free-dim
    ops there), then PE-transposed to [c, t] for the score matmuls.
  - scores are computed transposed: pT[s, t] = exp(scale * q_t . k_s). Since
    q/k are RMS-normalized, |score*scale| <= 8, so exp never overflows and no
    max-subtraction is needed.
  - softmax denominator comes from an all-ones [128,128] lhsT matmul over pT,
    which broadcasts the denominator across all 128 psum partitions so the
    later divide is a plain tensor_tensor multiply by a reciprocal.
  - PV matmul produces yT [j, t]; causality is handled by skipping fully
    masked s-blocks and narrowing matmuls + masking exp on diagonal blocks.
"""

import os
import sys

import numpy as np

for _p in ("/opt/trn_rl_repo", "/root/.axon_site/_ro/trn_rl_repo"):
    if os.path.isdir(_p) and _p not in sys.path:
        sys.path.insert(0, _p)

import ml_dtypes

import concourse.bass as bass
import concourse.mybir as mybir
import concourse.tile as tile
from concourse import bacc
from concourse.bass import ds, ts
from concourse.bass_utils import run_bass_kernel_spmd
from concourse.masks import make_identity

BF16 = mybir.dt.bfloat16
F32 = mybir.dt.float32
AF = mybir.ActivationFunctionType
ALU = mybir.AluOpType

N_HEAD = 16
D = 1024
HD = 64  # head dim
T = 2048
NCORES = 8
TB = T // 128  # 16 t-blocks
KC = D // 128  # 8 contraction chunks
NTC = T // 512  # 4 t-chunks of 512
LAMBDA_INIT = 0.8 - 0.6 * float(np.exp(-0.3 * 1))
EPS = float(np.finfo(np.float32).eps)
SCALE = 1.0 / 8.0  # 1/sqrt(64)

_CACHE = {}


def _build_program(lam: float) -> bass.Bass:
    nc = bacc.Bacc("TRN2", target_bir_lowering=False, debug=False)

    xT = nc.declare_dram_parameter("xT", [D, T], BF16, isOutput=False)
    wqkv = nc.declare_dram_parameter("wqkv", [D, 384], BF16, isOutput=False)
    wp = nc.declare_dram_parameter("wp", [D, 128], BF16, isOutput=False)
    cos_d = nc.declare_dram_parameter("cos", [128, TB * 32], BF16, isOutput=False)
    sin_d = nc.declare_dram_parameter("sin", [128, TB * 32], BF16, isOutput=False)
    diag_d = nc.declare_dram_parameter("diag", [128, 128], BF16, isOutput=False)
    outT = nc.declare_dram_parameter("outT", [128, T], F32, isOutput=True)

    with tile.TileContext(nc) as tc:
        with (
            tc.tile_pool(name="const", bufs=1) as cpool,
            tc.tile_pool(name="work", bufs=3) as wpool,
            tc.tile_pool(name="ptile", bufs=4) as ppool,
            tc.tile_pool(name="pp", bufs=2, space="PSUM") as pp_pool,
            tc.tile_pool(name="py", bufs=2, space="PSUM") as py_pool,
            tc.tile_pool(name="pd", bufs=2, space="PSUM") as pd_pool,
            tc.tile_pool(name="ptr", bufs=2, space="PSUM") as ptr_pool,
            tc.tile_pool(name="dram", bufs=1, space="DRAM") as dpool,
        ):
            # ---- persistent SBUF tensors ----
            xT_sb = cpool.tile([128, KC, T], BF16, tag="xT")
            wqkv_sb = cpool.tile([128, KC, 384], BF16, tag="wqkv")
            wp_sb = cpool.tile([128, KC, 128], BF16, tag="wp")
            cos_sb = cpool.tile([128, TB, 32], BF16, tag="cos")
            sin_sb = cpool.tile([128, TB, 32], BF16, tag="sin")
            diag_sb = cpool.tile([128, 128], BF16, tag="diag")
            ones_sb = cpool.tile([128, 128], BF16, tag="ones")
            ident_sb = cpool.tile([128, 128], BF16, tag="ident")
            qT_sb = cpool.tile([128, T], BF16, tag="qT")  # rows 0:64 g0, 64:128 g1
            kT_sb = cpool.tile([128, T], BF16, tag="kT")
            v_sb = cpool.tile([128, TB, 128], BF16, tag="v")  # [s-part, tb, j]
            ycomb_sb = cpool.tile([128, T], BF16, tag="ycomb")  # [j, t]
            yg_sb = cpool.tile([128, NCORES, T], BF16, tag="yg")  # gathered y
            out_sb = cpool.tile([128, T], F32, tag="out")

            # ---- load constants ----
            for kc in range(KC):
                nc.sync.dma_start(out=xT_sb[:, kc, :], in_=xT[ts(kc, 128), :])
                nc.sync.dma_start(out=wqkv_sb[:, kc, :], in_=wqkv[ts(kc, 128), :])
                nc.sync.dma_start(out=wp_sb[:, kc, :], in_=wp[ts(kc, 128), :])
            nc.sync.dma_start(
                out=cos_sb[:].rearrange("p a b -> p (a b)"), in_=cos_d[:, :]
            )
            nc.sync.dma_start(
                out=sin_sb[:].rearrange("p a b -> p (a b)"), in_=sin_d[:, :]
            )
            nc.sync.dma_start(out=diag_sb[:], in_=diag_d[:, :])
            nc.vector.memset(ones_sb[:], 1.0)
            make_identity(nc, ident_sb[:])
            eps_sb = cpool.tile([128, 1], F32, tag="eps")
            nc.vector.memset(eps_sb[:], EPS)

            # ---- stage B: QKV projection + rmsnorm + rotary + transpose ----
            for tb in range(TB):
                pqkv = pp_pool.tile([128, 384], F32, tag="pp")
                for kc in range(KC):
                    nc.tensor.matmul(
                        pqkv[:],
                        xT_sb[:, kc, ts(tb, 128)],
                        wqkv_sb[:, kc, :],
                        start=(kc == 0),
                        stop=(kc == KC - 1),
                    )
                # v slice -> v_sb (no norm)
                nc.scalar.copy(v_sb[:, tb, :], pqkv[:, 256:384])

                # sum of squares per 64-wide subhead (q1 q2 k1 k2)
                sq = wpool.tile([128, 256], F32, tag="sq")
                nc.scalar.square(sq[:], pqkv[:, 0:256])
                ssq = wpool.tile([128, 4], F32, tag="ssq")
                nc.vector.reduce_sum(
                    ssq[:], sq[:].rearrange("p (h c) -> p h c", c=HD), axis=mybir.AxisListType.X
                )
                # rscale = 1/sqrt(ssq/64 + eps)
                srt = wpool.tile([128, 4], F32, tag="srt")
                nc.scalar.activation(
                    srt[:], ssq[:], AF.Sqrt, bias=eps_sb[:], scale=1.0 / HD
                )
                rsc = wpool.tile([128, 4], F32, tag="rsc")
                nc.vector.reciprocal(rsc[:], srt[:])

                normed = wpool.tile([128, 4, HD], BF16, tag="normed")
                rscb = rsc[:].unsqueeze(2).broadcast_to([128, 4, HD])
                nc.vector.tensor_mul(
                    normed[:], pqkv[:, 0:256].rearrange("p (h c) -> p h c", c=HD), rscb
                )

                # rotary: out1 = n1*c + n2*s ; out2 = n2*c - n1*s
                n1 = normed[:, :, 0:32]
                n2 = normed[:, :, 32:64]
                cosb = cos_sb[:, tb, :].unsqueeze(1).broadcast_to([128, 4, 32])
                sinb = sin_sb[:, tb, :].unsqueeze(1).broadcast_to([128, 4, 32])
                rot = wpool.tile([128, 4, HD], BF16, tag="rot")
                tmp = wpool.tile([128, 4, 32], BF16, tag="rtmp")
                nc.vector.tensor_mul(tmp[:], n1, cosb)
                tmp2 = wpool.tile([128, 4, 32], BF16, tag="rtmp2")
                nc.vector.tensor_mul(tmp2[:], n2, sinb)
                nc.vector.tensor_add(rot[:, :, 0:32], tmp[:], tmp2[:])
                nc.vector.tensor_mul(tmp[:], n2, cosb)
                nc.vector.tensor_mul(tmp2[:], n1, sinb)
                nc.vector.tensor_sub(rot[:, :, 32:64], tmp[:], tmp2[:])

                # transpose q (subheads 0,1) and k (subheads 2,3) -> [c, t]
                rot2d = rot[:].rearrange("p a c -> p (a c)")
                ptq = ptr_pool.tile([128, 128], BF16, tag="ptr")
                nc.tensor.transpose(ptq[:], rot2d[:, 0:128], ident_sb[:])
                nc.scalar.copy(qT_sb[:, ts(tb, 128)], ptq[:])
                ptk = ptr_pool.tile([128, 128], BF16, tag="ptr")
                nc.tensor.transpose(ptk[:], rot2d[:, 128:256], ident_sb[:])
                nc.scalar.copy(kT_sb[:, ts(tb, 128)], ptk[:])

            # ---- stage C: differential attention, per group, per t-chunk ----
            y1n_tiles = {}
            for tc_i in range(NTC):
                nsb = 4 * tc_i + 4  # s-blocks touching this t-chunk
                for g in range(2):
                    py = py_pool.tile([128, 512], F32, tag="py")
                    pdn = pd_pool.tile([128, 512], F32, tag="pd")
                    for si in range(nsb):
                        col0 = max(0, si * 128 - tc_i * 512)
                        w = 512 - col0
                        pp = pp_pool.tile([128, 512], F32, tag="pp")
                        nc.tensor.matmul(
                            pp[:, col0:512],
                            kT_sb[ds(g * 64, 64), ts(si, 128)],
                            qT_sb[ds(g * 64, 64), ds(tc_i * 512 + col0, w)],
                            start=True,
                            stop=True,
                        )
                        pt = ppool.tile([128, 512], BF16, tag="pt")
                        nc.scalar.activation(
                            pt[:, col0:512], pp[:, col0:512], AF.Exp, scale=SCALE
                        )
                        if col0 > 0 or si * 128 == tc_i * 512:
                            # diagonal block: zero out s > t inside it
                            nc.vector.tensor_mul(
                                pt[:, col0 : col0 + 128],
                                pt[:, col0 : col0 + 128],
                                diag_sb[:],
                            )
                        nc.tensor.matmul(
                            py[:, col0:512],
                            v_sb[:, si, :],
                            pt[:, col0:512],
                            start=(si == 0),
                            stop=(si == nsb - 1),
                        )
                        nc.tensor.matmul(
                            pdn[:, col0:512],
                            ones_sb[:],
                            pt[:, col0:512],
                            start=(si == 0),
                            stop=(si == nsb - 1),
                        )
                    rec = wpool.tile([128, 512], F32, tag="rec")
                    nc.vector.reciprocal(rec[:], pdn[:])
                    if g == 0:
                        y1n = wpool.tile([128, 512], F32, tag="y1n")
                        nc.vector.tensor_mul(y1n[:], py[:], rec[:])
                        y1n_tiles[tc_i] = y1n
                    else:
                        y2n = wpool.tile([128, 512], F32, tag="y2n")
                        nc.vector.tensor_mul(y2n[:], py[:], rec[:])
                        nc.vector.scalar_tensor_tensor(
                            ycomb_sb[:, ts(tc_i, 512)],
                            y2n[:],
                            -lam,
                            y1n_tiles[tc_i][:],
                            ALU.mult,
                            ALU.add,
                        )

            # ---- stage D: AllGather y across the 8 cores ----
            y_dram = dpool.tile([128, T], BF16)
            yg_dram = dpool.tile([NCORES, 128, T], BF16, addr_space="Shared")
            nc.gpsimd.dma_start(y_dram[:], ycomb_sb[:])
            nc.gpsimd.collective_compute(
                "AllGather",
                ALU.bypass,
                replica_groups=[list(range(NCORES))],
                ins=[y_dram[:].opt()],
                outs=[yg_dram[:].opt()],
            )
            for h in range(NCORES):
                nc.sync.dma_start(out=yg_sb[:, h, :], in_=yg_dram[h, :, :])

            # ---- stage E: final projection (column shard) ----
            for tc_i in range(NTC):
                po = py_pool.tile([128, 512], F32, tag="py")
                for kc in range(KC):
                    nc.tensor.matmul(
                        po[:],
                        wp_sb[:, kc, :],
                        yg_sb[:, kc, ts(tc_i, 512)],
                        start=(kc == 0),
                        stop=(kc == KC - 1),
                    )
                nc.scalar.copy(out_sb[:, ts(tc_i, 512)], po[:])
            nc.sync.dma_start(out=outT[:, :], in_=out_sb[:])

    nc.compile()
    return nc


def kernel(x, Wq, Wk, Wv, Wproj, lambda_q1, lambda_k1, lambda_q2, lambda_k2):
    x = np.asarray(x, np.float32)
    Wq, Wk = np.asarray(Wq, np.float32), np.asarray(Wk, np.float32)
    Wv, Wproj = np.asarray(Wv, np.float32), np.asarray(Wproj, np.float32)

    lam1 = float(np.exp(np.sum(np.asarray(lambda_q1) * np.asarray(lambda_k1))))
    lam2 = float(np.exp(np.sum(np.asarray(lambda_q2) * np.asarray(lambda_k2))))
    lam = lam1 - lam2 + LAMBDA_INIT

    bf = ml_dtypes.bfloat16
    xT = np.ascontiguousarray(x[0].T).astype(bf)  # [D, T]

    # rotary tables, rearranged to [tp, tb, 32] and flattened
    inv = 1.0 / (10000.0 ** (np.arange(0, HD, 2, dtype=np.float32) / HD))
    fr = np.outer(np.arange(T, dtype=np.float32), inv)  # [T, 32]
    cos = np.cos(fr).reshape(TB, 128, 32).transpose(1, 0, 2).reshape(128, -1)
    sin = np.sin(fr).reshape(TB, 128, 32).transpose(1, 0, 2).reshape(128, -1)
    cos, sin = cos.astype(bf), sin.astype(bf)
    diag = np.triu(np.ones((128, 128), np.float32)).astype(bf)

    in_maps = []
    for h in range(NCORES):
        wqk = np.concatenate(
            [
                Wq[h * 64 : h * 64 + 64],
                Wq[512 + h * 64 : 512 + h * 64 + 64],
                Wk[h * 64 : h * 64 + 64],
                Wk[512 + h * 64 : 512 + h * 64 + 64],
                Wv[h * 128 : h * 128 + 128],
            ],
            axis=0,
        ).T  # [D, 384]
        wp = Wproj[h * 128 : h * 128 + 128, :].T  # [D(j), 128(i)]
        in_maps.append(
            {
                "xT": xT,
                "wqkv": np.ascontiguousarray(wqk).astype(bf),
                "wp": np.ascontiguousarray(wp).astype(bf),
                "cos": cos,
                "sin": sin,
                "diag": diag,
            }
        )

    key = round(lam, 10)
    if key not in _CACHE:
        _CACHE[key] = _build_program(lam)
    nc = _CACHE[key]

    res = run_bass_kernel_spmd(nc, in_maps, list(range(NCORES)))
    out = np.empty((T, D), np.float32)
    for h in range(NCORES):
        out[:, h * 128 : (h + 1) * 128] = res.results[h]["outT"].T
    return out.reshape(1, T, D)


if __name__ == "__main__":
    rng = np.random.default_rng(0)
    ins = {
        "x": rng.standard_normal((1, T, D), np.float32),
        "Wq": (rng.standard_normal((D, D)) * 0.02).astype(np.float32),
        "Wk": (rng.standard_normal((D, D)) * 0.02).astype(np.float32),
        "Wv": (rng.standard_normal((D, D)) * 0.02).astype(np.float32),
        "Wproj": (rng.standard_normal((D, D)) * 0.02).astype(np.float32),
        "lambda_q1": (rng.standard_normal(32) * 0.1).astype(np.float32),
        "lambda_k1": (rng.standard_normal(32) * 0.1).astype(np.float32),
        "lambda_q2": (rng.standard_normal(32) * 0.1).astype(np.float32),
        "lambda_k2": (rng.standard_normal(32) * 0.1).astype(np.float32),
    }
    y = kernel(**ins)
    print("kernel output", y.shape, y.dtype, float(np.abs(y).mean()))


# revision 24
# speedup vs baseline: 1.5123x; 1.5123x over previous
"""Trainium2 Bass kernel for MixerDiffAttention (differential attention).

Sharding: tensor-parallel over the 8 (n_head//2) head groups across 8 cores
(data-parallel over B is trivial since B=1). Each core computes the QKV
projections for its head group, both differential attention branches, the
normalized combination y1 - lambda*y2, and its head's partial product with
the row-sharded c_proj. The host sums the 8 partial outputs (the unshard
step for row-parallel tensor parallelism).

Math layout notes (per core, head h):
  - x is host-transposed to xT [D, T] so D (the contraction dim) sits on
    SBUF partitions for all projection matmuls.
  - q/k are produced in [t, c] layout (rmsnorm reduce + rotary are free-dim
    ops there), then PE-transposed to [c, t] for the score matmuls.
  - scores are computed transposed: pT[s, t] = exp(scale * q_t . k_s). Since
    q/k are RMS-normalized, |score*scale| <= 8, so exp never overflows and no
    max-subtraction is needed.
  - softmax denominator comes from an all-ones [128,128] lhsT matmul over pT,
    which broadcasts the denominator across all 128 psum partitions so the
    divide is a plain tensor_tensor multiply by a reciprocal.
  - PV matmul produces yT [j, t]; causality is handled by skipping fully
    masked s-blocks, narrowing matmuls on diagonal blocks, and masking exp
    output on the diagonal.
  - The partial projection for t-chunk tc depends only on ycomb[:, tc], so
    Tile overlaps it with the attention of later t-chunks.
"""

import os
import sys

import numpy as np

for _p in ("/opt/trn_rl_repo", "/root/.axon_site/_ro/trn_rl_repo"):
    if os.path.isdir(_p) and _p not in sys.path:
        sys.path.insert(0, _p)

import ml_dtypes

import concourse.bass as bass
import concourse.mybir as mybir
import concourse.tile as tile
from concourse import bacc
from concourse.bass import ds, ts
from concourse.bass_utils import run_bass_kernel_spmd
from concourse.masks import make_identity

BF16 = mybir.dt.bfloat16
F32 = mybir.dt.float32
AF = mybir.ActivationFunctionType
ALU = mybir.AluOpType

N_HEAD = 16
D = 1024
HD = 64  # head dim
T = 2048
NCORES = 8
TB = T // 128  # 16 t-blocks
KC = D // 128  # 8 contraction chunks
NTC = T // 512  # 4 t-chunks of 512
LAMBDA_INIT = 0.8 - 0.6 * float(np.exp(-0.3 * 1))
EPS = float(np.finfo(np.float32).eps)
SCALE = 1.0 / 8.0  # 1/sqrt(64)

_CACHE = {}


def _build_program(lam: float) -> bass.Bass:
    nc = bacc.Bacc("TRN2", target_bir_lowering=False, debug=False)

    xT = nc.declare_dram_parameter("xT", [D, T], BF16, isOutput=False)
    wqkv = nc.declare_dram_parameter("wqkv", [D, 384], BF16, isOutput=False)
    wpp = nc.declare_dram_parameter("wpp", [128, D], BF16, isOutput=False)
    cos_d = nc.declare_dram_parameter("cos", [128, TB * 32], BF16, isOutput=False)
    sin_d = nc.declare_dram_parameter("sin", [128, TB * 32], BF16, isOutput=False)
    diag_d = nc.declare_dram_parameter("diag", [128, 128], BF16, isOutput=False)
    outTp = nc.declare_dram_parameter("outTp", [D, T], F32, isOutput=True)

    with tile.TileContext(nc) as tc:
        with (
            tc.tile_pool(name="const", bufs=1) as cpool,
            tc.tile_pool(name="work", bufs=3) as wpool,
            tc.tile_pool(name="ptile", bufs=4) as ppool,
            tc.tile_pool(name="ostage", bufs=3) as opool,
            tc.tile_pool(name="pp", bufs=2, space="PSUM") as pp_pool,
            tc.tile_pool(name="py", bufs=2, space="PSUM") as py_pool,
            tc.tile_pool(name="pd", bufs=2, space="PSUM") as pd_pool,
            tc.tile_pool(name="ptr", bufs=2, space="PSUM") as ptr_pool,
        ):
            # ---- persistent SBUF tensors ----
            xT_sb = cpool.tile([128, KC, T], BF16, tag="xT")
            wqkv_sb = cpool.tile([128, KC, 384], BF16, tag="wqkv")
            wpp_sb = cpool.tile([128, KC, 128], BF16, tag="wpp")
            cos_sb = cpool.tile([128, TB, 32], BF16, tag="cos")
            sin_sb = cpool.tile([128, TB, 32], BF16, tag="sin")
            diag_sb = cpool.tile([128, 128], BF16, tag="diag")
            ones_sb = cpool.tile([128, 128], BF16, tag="ones")
            ident_sb = cpool.tile([128, 128], BF16, tag="ident")
            qT_sb = cpool.tile([128, T], BF16, tag="qT")  # rows 0:64 g0, 64:128 g1
            kT_sb = cpool.tile([128, T], BF16, tag="kT")
            v_sb = cpool.tile([128, TB, 128], BF16, tag="v")  # [s-part, tb, j]
            ycomb_sb = cpool.tile([128, T], BF16, tag="ycomb")  # [j, t]

            # ---- load constants ----
            for kc in range(KC):
                nc.sync.dma_start(out=wqkv_sb[:, kc, :], in_=wqkv[ts(kc, 128), :])
                nc.sync.dma_start(
                    out=wpp_sb[:, kc, :], in_=wpp[:, ts(kc, 128)]
                )
            # x chunks split along t so early t-blocks land first
            for tc_i in range(NTC):
                for kc in range(KC):
                    nc.sync.dma_start(
                        out=xT_sb[:, kc, ts(tc_i, 512)],
                        in_=xT[ts(kc, 128), ts(tc_i, 512)],
                    )
            nc.sync.dma_start(
                out=cos_sb[:].rearrange("p a b -> p (a b)"), in_=cos_d[:, :]
            )
            nc.sync.dma_start(
                out=sin_sb[:].rearrange("p a b -> p (a b)"), in_=sin_d[:, :]
            )
            nc.sync.dma_start(out=diag_sb[:], in_=diag_d[:, :])
            nc.vector.memset(ones_sb[:], 1.0)
            make_identity(nc, ident_sb[:])
            eps_sb = cpool.tile([128, 1], F32, tag="eps")
            nc.vector.memset(eps_sb[:], EPS)

            # ---- stage B: QKV projection + rmsnorm + rotary + transpose ----
            for tb in range(TB):
                pqkv = pp_pool.tile([128, 384], F32, tag="pp")
                for kc in range(KC):
                    nc.tensor.matmul(
                        pqkv[:],
                        xT_sb[:, kc, ts(tb, 128)],
                        wqkv_sb[:, kc, :],
                        start=(kc == 0),
                        stop=(kc == KC - 1),
                    )
                # v slice -> v_sb (no norm)
                nc.scalar.copy(v_sb[:, tb, :], pqkv[:, 256:384])

                # sum of squares per 64-wide subhead (q1 q2 k1 k2)
                sq = wpool.tile([128, 256], F32, tag="sq")
                nc.scalar.square(sq[:], pqkv[:, 0:256])
                ssq = wpool.tile([128, 4], F32, tag="ssq")
                nc.vector.reduce_sum(
                    ssq[:],
                    sq[:].rearrange("p (h c) -> p h c", c=HD),
                    axis=mybir.AxisListType.X,
                )
                # rscale = 1/sqrt(ssq/64 + eps)
                srt = wpool.tile([128, 4], F32, tag="srt")
                nc.scalar.activation(
                    srt[:], ssq[:], AF.Sqrt, bias=eps_sb[:], scale=1.0 / HD
                )
                rsc = wpool.tile([128, 4], F32, tag="rsc")
                nc.vector.reciprocal(rsc[:], srt[:])

                normed = wpool.tile([128, 4, HD], BF16, tag="normed")
                rscb = rsc[:].unsqueeze(2).broadcast_to([128, 4, HD])
                nc.vector.tensor_mul(
                    normed[:],
                    pqkv[:, 0:256].rearrange("p (h c) -> p h c", c=HD),
                    rscb,
                )

                # rotary: out1 = n1*c + n2*s ; out2 = n2*c - n1*s
                n1 = normed[:, :, 0:32]
                n2 = normed[:, :, 32:64]
                cosb = cos_sb[:, tb, :].unsqueeze(1).broadcast_to([128, 4, 32])
                sinb = sin_sb[:, tb, :].unsqueeze(1).broadcast_to([128, 4, 32])
                rot = wpool.tile([128, 4, HD], BF16, tag="rot")
                tmp = wpool.tile([128, 4, 32], BF16, tag="rtmp")
                nc.vector.tensor_mul(tmp[:], n1, cosb)
                tmp2 = wpool.tile([128, 4, 32], BF16, tag="rtmp2")
                nc.vector.tensor_mul(tmp2[:], n2, sinb)
                nc.vector.tensor_add(rot[:, :, 0:32], tmp[:], tmp2[:])
                nc.vector.tensor_mul(tmp[:], n2, cosb)
                nc.vector.tensor_mul(tmp2[:], n1, sinb)
                nc.vector.tensor_sub(rot[:, :, 32:64], tmp[:], tmp2[:])

                # transpose q (subheads 0,1) and k (subheads 2,3) -> [c, t]
                rot2d = rot[:].rearrange("p a c -> p (a c)")
                ptq = ptr_pool.tile([128, 128], BF16, tag="ptr")
                nc.tensor.transpose(ptq[:], rot2d[:, 0:128], ident_sb[:])
                nc.scalar.copy(qT_sb[:, ts(tb, 128)], ptq[:])
                ptk = ptr_pool.tile([128, 128], BF16, tag="ptr")
                nc.tensor.transpose(ptk[:], rot2d[:, 128:256], ident_sb[:])
                nc.scalar.copy(kT_sb[:, ts(tb, 128)], ptk[:])

            # ---- stage C: differential attention + partial projection ----
            y1n_tiles = {}
            for tc_i in range(NTC):
                nsb = 4 * tc_i + 4  # s-blocks touching this t-chunk
                for g in range(2):
                    py = py_pool.tile([128, 512], F32, tag="py")
                    pdn = pd_pool.tile([128, 512], F32, tag="pd")
                    for si in range(nsb):
                        col0 = max(0, si * 128 - tc_i * 512)
                        w = 512 - col0
                        pp = pp_pool.tile([128, 512], F32, tag="pp")
                        nc.tensor.matmul(
                            pp[:, col0:512],
                            kT_sb[ds(g * 64, 64), ts(si, 128)],
                            qT_sb[ds(g * 64, 64), ds(tc_i * 512 + col0, w)],
                            start=True,
                            stop=True,
                        )
                        pt = ppool.tile([128, 512], BF16, tag="pt")
                        nc.scalar.activation(
                            pt[:, col0:512], pp[:, col0:512], AF.Exp, scale=SCALE
                        )
                        if col0 > 0 or si * 128 == tc_i * 512:
                            # diagonal block: zero out s > t inside it
                            nc.vector.tensor_mul(
                                pt[:, col0 : col0 + 128],
                                pt[:, col0 : col0 + 128],
                                diag_sb[:],
                            )
                        nc.tensor.matmul(
                            py[:, col0:512],
                            v_sb[:, si, :],
                            pt[:, col0:512],
                            start=(si == 0),
                            stop=(si == nsb - 1),
                        )
                        nc.tensor.matmul(
                            pdn[:, col0:512],
                            ones_sb[:],
                            pt[:, col0:512],
                            start=(si == 0),
                            stop=(si == nsb - 1),
                        )
                    rec = wpool.tile([128, 512], F32, tag="rec")
                    nc.vector.reciprocal_approx_fast(rec[:], pdn[:])
                    if g == 0:
                        y1n = wpool.tile([128, 512], F32, tag="y1n")
                        nc.vector.tensor_mul(y1n[:], py[:], rec[:])
                        y1n_tiles[tc_i] = y1n
                    else:
                        y2n = wpool.tile([128, 512], F32, tag="y2n")
                        nc.vector.tensor_mul(y2n[:], py[:], rec[:])
                        nc.vector.scalar_tensor_tensor(
                            ycomb_sb[:, ts(tc_i, 512)],
                            y2n[:],
                            -lam,
                            y1n_tiles[tc_i][:],
                            ALU.mult,
                            ALU.add,
                        )

                # partial projection for this t-chunk: overlaps later chunks
                for ic in range(KC):
                    po = ptr_pool.tile([128, 512], F32, tag="ptr")
                    nc.tensor.matmul(
                        po[:],
                        wpp_sb[:, ic, :],
                        ycomb_sb[:, ts(tc_i, 512)],
                        start=True,
                        stop=True,
                    )
                    ost = opool.tile([128, 512], F32, tag="ost")
                    nc.scalar.copy(ost[:], po[:])
                    nc.sync.dma_start(
                        out=outTp[ts(ic, 128), ts(tc_i, 512)], in_=ost[:]
                    )

    nc.compile()
    return nc


def _make_in_maps(x, Wq, Wk, Wv, Wproj):
    bf = ml_dtypes.bfloat16
    xT = np.ascontiguousarray(x[0].T).astype(bf)  # [D, T]

    # rotary tables, rearranged to [tp, tb, 32] and flattened
    inv = 1.0 / (10000.0 ** (np.arange(0, HD, 2, dtype=np.float32) / HD))
    fr = np.outer(np.arange(T, dtype=np.float32), inv)  # [T, 32]
    cos = np.cos(fr).reshape(TB, 128, 32).transpose(1, 0, 2).reshape(128, -1)
    sin = np.sin(fr).reshape(TB, 128, 32).transpose(1, 0, 2).reshape(128, -1)
    cos, sin = cos.astype(bf), sin.astype(bf)
    diag = np.triu(np.ones((128, 128), np.float32)).astype(bf)

    in_maps = []
    for h in range(NCORES):
        wqk = np.concatenate(
            [
                Wq[h * 64 : h * 64 + 64],
                Wq[512 + h * 64 : 512 + h * 64 + 64],
                Wk[h * 64 : h * 64 + 64],
                Wk[512 + h * 64 : 512 + h * 64 + 64],
                Wv[h * 128 : h * 128 + 128],
            ],
            axis=0,
        ).T  # [D, 384]
        # wpp[j, i] = Wproj[i, h*128+j] -- lhsT chunks for the partial proj
        wpp = Wproj[:, h * 128 : (h + 1) * 128].T  # [128 j, 1024 i]
        in_maps.append(
            {
                "xT": xT,
                "wqkv": np.ascontiguousarray(wqk).astype(bf),
                "wpp": np.ascontiguousarray(wpp).astype(bf),
                "cos": cos,
                "sin": sin,
                "diag": diag,
            }
        )
    return in_maps


def _get_program(lam: float):
    key = round(lam, 10)
    if key not in _CACHE:
        _CACHE[key] = _build_program(lam)
    return _CACHE[key]


def kernel(x, Wq, Wk, Wv, Wproj, lambda_q1, lambda_k1, lambda_q2, lambda_k2):
    x = np.asarray(x, np.float32)
    Wq, Wk = np.asarray(Wq, np.float32), np.asarray(Wk, np.float32)
    Wv, Wproj = np.asarray(Wv, np.float32), np.asarray(Wproj, np.float32)

    lam1 = float(np.exp(np.sum(np.asarray(lambda_q1) * np.asarray(lambda_k1))))
    lam2 = float(np.exp(np.sum(np.asarray(lambda_q2) * np.asarray(lambda_k2))))
    lam = lam1 - lam2 + LAMBDA_INIT

    in_maps = _make_in_maps(x, Wq, Wk, Wv, Wproj)
    nc = _get_program(lam)

    res = run_bass_kernel_spmd(nc, in_maps, list(range(NCORES)))
    # unshard: row-parallel c_proj -> sum the 8 partial products
    acc = res.results[0]["outTp"].astype(np.float64)
    for h in range(1, NCORES):
        acc += res.results[h]["outTp"]
    return np.ascontiguousarray(acc.T).astype(np.float32).reshape(1, T, D)


if __name__ == "__main__":
    rng = np.random.default_rng(0)
    ins = {
        "x": rng.standard_normal((1, T, D), np.float32),
        "Wq": (rng.standard_normal((D, D)) * 0.02).astype(np.float32),
        "Wk": (rng.standard_normal((D, D)) * 0.02).astype(np.float32),
        "Wv": (rng.standard_normal((D, D)) * 0.02).astype(np.float32),
        "Wproj": (rng.standard_normal((D, D)) * 0.02).astype(np.float32),
        "lambda_q1": (rng.standard_normal(32) * 0.1).astype(np.float32),
        "lambda_k1": (rng.standard_normal(32) * 0.1).astype(np.float32),
        "lambda_q2": (rng.standard_normal(32) * 0.1).astype(np.float32),
        "lambda_k2": (rng.standard_normal(32) * 0.1).astype(np.float32),
    }
    y = kernel(**ins)
    print("kernel output", y.shape, y.dtype, float(np.abs(y).mean()))


# revision 28
# speedup vs baseline: 1.6659x; 1.1016x over previous
"""Trainium2 Bass kernel for MixerDiffAttention (differential attention).

Sharding: tensor-parallel over the 8 (n_head//2) head groups across 8 cores
(data-parallel over B is trivial since B=1). Each core computes the QKV
projections for its head group, both differential attention branches, the
normalized combination y1 - lambda*y2, and its head's partial product with
the row-sharded c_proj. The host sums the 8 partial outputs (the unshard
step for row-parallel tensor parallelism).

Math layout notes (per core, head h):
  - x is host-transposed to xT [D, T] so D (the contraction dim) sits on
    SBUF partitions for all projection matmuls.
  - q/k are produced in [t, c] layout (rmsnorm reduce + rotary are free-dim
    ops there), then PE-transposed to [c, t] for the score matmuls.
  - scores are computed transposed: pT[s, t] = exp(scale * q_t . k_s). Since
    q/k are RMS-normalized, |score*scale| <= 8, so exp never overflows and no
    max-subtraction is needed.
  - softmax denominator comes from an all-ones [128,128] lhsT matmul over pT,
    which broadcasts the denominator across all 128 psum partitions so the
    divide is a plain tensor_tensor multiply by a reciprocal.
  - PV matmul produces yT [j, t]; causality is handled by skipping fully
    masked s-blocks, narrowing matmuls on diagonal blocks, and masking exp
    output on the diagonal.
  - The partial projection for t-chunk tc depends only on ycomb[:, tc], so
    Tile overlaps it with the attention of later t-chunks.
"""

import os
import sys

import numpy as np

for _p in ("/opt/trn_rl_repo", "/root/.axon_site/_ro/trn_rl_repo"):
    if os.path.isdir(_p) and _p not in sys.path:
        sys.path.insert(0, _p)

import ml_dtypes

import concourse.bass as bass
import concourse.mybir as mybir
import concourse.tile as tile
from concourse import bacc
from concourse.bass import ds, ts
from concourse.bass_utils import run_bass_kernel_spmd
from concourse.masks import make_identity

BF16 = mybir.dt.bfloat16
F32 = mybir.dt.float32
AF = mybir.ActivationFunctionType
ALU = mybir.AluOpType

N_HEAD = 16
D = 1024
HD = 64  # head dim
T = 2048
NCORES = 8
TB = T // 128  # 16 t-blocks
KC = D // 128  # 8 contraction chunks
NTC = T // 512  # 4 t-chunks of 512
LAMBDA_INIT = 0.8 - 0.6 * float(np.exp(-0.3 * 1))
EPS = float(np.finfo(np.float32).eps)
SCALE = 1.0 / 8.0  # 1/sqrt(64)

_CACHE = {}


def _build_program(lam: float) -> bass.Bass:
    nc = bacc.Bacc("TRN2", target_bir_lowering=False, debug=False)

    xT = nc.declare_dram_parameter("xT", [D, T], BF16, isOutput=False)
    wqkv = nc.declare_dram_parameter("wqkv", [D, 384], BF16, isOutput=False)
    wpp = nc.declare_dram_parameter("wpp", [128, D], BF16, isOutput=False)
    cos_d = nc.declare_dram_parameter("cos", [128, TB * 32], BF16, isOutput=False)
    sin_d = nc.declare_dram_parameter("sin", [128, TB * 32], BF16, isOutput=False)
    diag_d = nc.declare_dram_parameter("diag", [128, 128], BF16, isOutput=False)
    outTp = nc.declare_dram_parameter("outTp", [D, T], F32, isOutput=True)

    with tile.TileContext(nc) as tc:
        with (
            tc.tile_pool(name="const", bufs=1) as cpool,
            tc.tile_pool(name="work", bufs=3) as wpool,
            tc.tile_pool(name="ptile", bufs=4) as ppool,
            tc.tile_pool(name="ostage", bufs=3) as opool,
            tc.tile_pool(name="pp", bufs=2, space="PSUM") as pp_pool,
            tc.tile_pool(name="py", bufs=2, space="PSUM") as py_pool,
            tc.tile_pool(name="pd", bufs=2, space="PSUM") as pd_pool,
            tc.tile_pool(name="ptr", bufs=2, space="PSUM") as ptr_pool,
        ):
            # ---- persistent SBUF tensors ----
            xT_sb = cpool.tile([128, KC, T], BF16, tag="xT")
            wqkv_sb = cpool.tile([128, KC, 384], BF16, tag="wqkv")
            wpp_sb = cpool.tile([128, KC, 128], BF16, tag="wpp")
            cos_sb = cpool.tile([128, TB, 32], BF16, tag="cos")
            sin_sb = cpool.tile([128, TB, 32], BF16, tag="sin")
            diag_sb = cpool.tile([128, 128], BF16, tag="diag")
            ones_sb = cpool.tile([128, 128], BF16, tag="ones")
            ident_sb = cpool.tile([128, 128], BF16, tag="ident")
            qT_sb = cpool.tile([128, T], BF16, tag="qT")  # rows 0:64 g0, 64:128 g1
            kT_sb = cpool.tile([128, T], BF16, tag="kT")
            v_sb = cpool.tile([128, TB, 128], BF16, tag="v")  # [s-part, tb, j]
            ycomb_sb = cpool.tile([128, T], BF16, tag="ycomb")  # [j, t]

            # ---- load constants ----
            for kc in range(KC):
                nc.sync.dma_start(out=wqkv_sb[:, kc, :], in_=wqkv[ts(kc, 128), :])
                nc.sync.dma_start(
                    out=wpp_sb[:, kc, :], in_=wpp[:, ts(kc, 128)]
                )
            # x chunks split along t so early t-blocks land first; alternate
            # HWDGE (sync) and SWDGE (gpsimd) rings to double DMA issue rate
            for tc_i in range(NTC):
                for kc in range(KC):
                    eng = nc.sync if kc % 2 == 0 else nc.gpsimd
                    eng.dma_start(
                        out=xT_sb[:, kc, ts(tc_i, 512)],
                        in_=xT[ts(kc, 128), ts(tc_i, 512)],
                    )
            nc.sync.dma_start(
                out=cos_sb[:].rearrange("p a b -> p (a b)"), in_=cos_d[:, :]
            )
            nc.sync.dma_start(
                out=sin_sb[:].rearrange("p a b -> p (a b)"), in_=sin_d[:, :]
            )
            nc.sync.dma_start(out=diag_sb[:], in_=diag_d[:, :])
            nc.vector.memset(ones_sb[:], 1.0)
            make_identity(nc, ident_sb[:])
            eps_sb = cpool.tile([128, 1], F32, tag="eps")
            nc.vector.memset(eps_sb[:], EPS)

            # ---- stage B: QKV projection + rmsnorm + rotary + transpose ----
            for tb in range(TB):
                pqkv = pp_pool.tile([128, 384], F32, tag="pp")
                for kc in range(KC):
                    nc.tensor.matmul(
                        pqkv[:],
                        xT_sb[:, kc, ts(tb, 128)],
                        wqkv_sb[:, kc, :],
                        start=(kc == 0),
                        stop=(kc == KC - 1),
                    )
                # v slice -> v_sb (no norm)
                nc.vector.tensor_copy(v_sb[:, tb, :], pqkv[:, 256:384])

                # sum of squares per 64-wide subhead (q1 q2 k1 k2)
                sq = wpool.tile([128, 256], F32, tag="sq")
                nc.scalar.square(sq[:], pqkv[:, 0:256])
                ssq = wpool.tile([128, 4], F32, tag="ssq")
                nc.vector.reduce_sum(
                    ssq[:],
                    sq[:].rearrange("p (h c) -> p h c", c=HD),
                    axis=mybir.AxisListType.X,
                )
                # rscale = 1/sqrt(ssq/64 + eps)
                srt = wpool.tile([128, 4], F32, tag="srt")
                nc.scalar.activation(
                    srt[:], ssq[:], AF.Sqrt, bias=eps_sb[:], scale=1.0 / HD
                )
                rsc = wpool.tile([128, 4], F32, tag="rsc")
                nc.vector.reciprocal(rsc[:], srt[:])

                normed = wpool.tile([128, 4, HD], BF16, tag="normed")
                rscb = rsc[:].unsqueeze(2).broadcast_to([128, 4, HD])
                nc.vector.tensor_mul(
                    normed[:],
                    pqkv[:, 0:256].rearrange("p (h c) -> p h c", c=HD),
                    rscb,
                )

                # rotary: out1 = n1*c + n2*s ; out2 = n2*c - n1*s
                n1 = normed[:, :, 0:32]
                n2 = normed[:, :, 32:64]
                cosb = cos_sb[:, tb, :].unsqueeze(1).broadcast_to([128, 4, 32])
                sinb = sin_sb[:, tb, :].unsqueeze(1).broadcast_to([128, 4, 32])
                rot = wpool.tile([128, 4, HD], BF16, tag="rot")
                tmp = wpool.tile([128, 4, 32], BF16, tag="rtmp")
                nc.vector.tensor_mul(tmp[:], n1, cosb)
                tmp2 = wpool.tile([128, 4, 32], BF16, tag="rtmp2")
                nc.vector.tensor_mul(tmp2[:], n2, sinb)
                nc.vector.tensor_add(rot[:, :, 0:32], tmp[:], tmp2[:])
                nc.vector.tensor_mul(tmp[:], n2, cosb)
                nc.vector.tensor_mul(tmp2[:], n1, sinb)
                nc.vector.tensor_sub(rot[:, :, 32:64], tmp[:], tmp2[:])

                # transpose q (subheads 0,1) and k (subheads 2,3) -> [c, t]
                rot2d = rot[:].rearrange("p a c -> p (a c)")
                ptq = ptr_pool.tile([128, 128], BF16, tag="ptr")
                nc.tensor.transpose(ptq[:], rot2d[:, 0:128], ident_sb[:])
                nc.scalar.copy(qT_sb[:, ts(tb, 128)], ptq[:])
                ptk = ptr_pool.tile([128, 128], BF16, tag="ptr")
                nc.tensor.transpose(ptk[:], rot2d[:, 128:256], ident_sb[:])
                nc.scalar.copy(kT_sb[:, ts(tb, 128)], ptk[:])

            # ---- stage C: differential attention + partial projection ----
            y1n_tiles = {}
            for tc_i in range(NTC):
                nsb = 4 * tc_i + 4  # s-blocks touching this t-chunk
                for g in range(2):
                    py = py_pool.tile([128, 512], F32, tag="py")
                    pdn = pd_pool.tile([128, 512], F32, tag="pd")
                    for si in range(nsb):
                        col0 = max(0, si * 128 - tc_i * 512)
                        w = 512 - col0
                        pp = pp_pool.tile([128, 512], F32, tag="pp")
                        nc.tensor.matmul(
                            pp[:, col0:512],
                            kT_sb[ds(g * 64, 64), ts(si, 128)],
                            qT_sb[ds(g * 64, 64), ds(tc_i * 512 + col0, w)],
                            start=True,
                            stop=True,
                        )
                        pt = ppool.tile([128, 512], BF16, tag="pt")
                        nc.scalar.activation(
                            pt[:, col0:512], pp[:, col0:512], AF.Exp, scale=SCALE
                        )
                        if col0 > 0 or si * 128 == tc_i * 512:
                            # diagonal block: zero out s > t inside it
                            nc.vector.tensor_mul(
                                pt[:, col0 : col0 + 128],
                                pt[:, col0 : col0 + 128],
                                diag_sb[:],
                            )
                        nc.tensor.matmul(
                            py[:, col0:512],
                            v_sb[:, si, :],
                            pt[:, col0:512],
                            start=(si == 0),
                            stop=(si == nsb - 1),
                        )
                        nc.tensor.matmul(
                            pdn[:, col0:512],
                            ones_sb[:],
                            pt[:, col0:512],
                            start=(si == 0),
                            stop=(si == nsb - 1),
                        )
                    rec = wpool.tile([128, 512], F32, tag="rec")
                    nc.vector.reciprocal_approx_fast(rec[:], pdn[:])
                    if g == 0:
                        y1n = wpool.tile([128, 512], F32, tag="y1n")
                        nc.vector.tensor_mul(y1n[:], py[:], rec[:])
                        y1n_tiles[tc_i] = y1n
                    else:
                        y2n = wpool.tile([128, 512], F32, tag="y2n")
                        nc.vector.tensor_mul(y2n[:], py[:], rec[:])
                        nc.vector.scalar_tensor_tensor(
                            ycomb_sb[:, ts(tc_i, 512)],
                            y2n[:],
                            -lam,
                            y1n_tiles[tc_i][:],
                            ALU.mult,
                            ALU.add,
                        )

                # partial projection for this t-chunk: overlaps later chunks
                for ic in range(KC):
                    po = ptr_pool.tile([128, 512], F32, tag="ptr")
                    nc.tensor.matmul(
                        po[:],
                        wpp_sb[:, ic, :],
                        ycomb_sb[:, ts(tc_i, 512)],
                        start=True,
                        stop=True,
                    )
                    ost = opool.tile([128, 512], F32, tag="ost")
                    nc.vector.tensor_copy(ost[:], po[:])
                    nc.sync.dma_start(
                        out=outTp[ts(ic, 128), ts(tc_i, 512)], in_=ost[:]
                    )

    nc.compile()
    return nc


def _make_in_maps(x, Wq, Wk, Wv, Wproj):
    bf = ml_dtypes.bfloat16
    xT = np.ascontiguousarray(x[0].T).astype(bf)  # [D, T]

    # rotary tables, rearranged to [tp, tb, 32] and flattened
    inv = 1.0 / (10000.0 ** (np.arange(0, HD, 2, dtype=np.float32) / HD))
    fr = np.outer(np.arange(T, dtype=np.float32), inv)  # [T, 32]
    cos = np.cos(fr).reshape(TB, 128, 32).transpose(1, 0, 2).reshape(128, -1)
    sin = np.sin(fr).reshape(TB, 128, 32).transpose(1, 0, 2).reshape(128, -1)
    cos, sin = cos.astype(bf), sin.astype(bf)
    diag = np.triu(np.ones((128, 128), np.float32)).astype(bf)

    in_maps = []
    for h in range(NCORES):
        wqk = np.concatenate(
            [
                Wq[h * 64 : h * 64 + 64],
                Wq[512 + h * 64 : 512 + h * 64 + 64],
                Wk[h * 64 : h * 64 + 64],
                Wk[512 + h * 64 : 512 + h * 64 + 64],
                Wv[h * 128 : h * 128 + 128],
            ],
            axis=0,
        ).T  # [D, 384]
        # wpp[j, i] = Wproj[i, h*128+j] -- lhsT chunks for the partial proj
        wpp = Wproj[:, h * 128 : (h + 1) * 128].T  # [128 j, 1024 i]
        in_maps.append(
            {
                "xT": xT,
                "wqkv": np.ascontiguousarray(wqk).astype(bf),
                "wpp": np.ascontiguousarray(wpp).astype(bf),
                "cos": cos,
                "sin": sin,
                "diag": diag,
            }
        )
    return in_maps


def _get_program(lam: float):
    key = round(lam, 10)
    if key not in _CACHE:
        _CACHE[key] = _build_program(lam)
    return _CACHE[key]


def kernel(x, Wq, Wk, Wv, Wproj, lambda_q1, lambda_k1, lambda_q2, lambda_k2):
    x = np.asarray(x, np.float32)
    Wq, Wk = np.asarray(Wq, np.float32), np.asarray(Wk, np.float32)
    Wv, Wproj = np.asarray(Wv, np.float32), np.asarray(Wproj, np.float32)

    lam1 = float(np.exp(np.sum(np.asarray(lambda_q1) * np.asarray(lambda_k1))))
    lam2 = float(np.exp(np.sum(np.asarray(lambda_q2) * np.asarray(lambda_k2))))
    lam = lam1 - lam2 + LAMBDA_INIT

    in_maps = _make_in_maps(x, Wq, Wk, Wv, Wproj)
    nc = _get_program(lam)

    res = run_bass_kernel_spmd(nc, in_maps, list(range(NCORES)))
    # unshard: row-parallel c_proj -> sum the 8 partial products
    acc = res.results[0]["outTp"].astype(np.float64)
    for h in range(1, NCORES):
        acc += res.results[h]["outTp"]
    return np.ascontiguousarray(acc.T).astype(np.float32).reshape(1, T, D)


if __name__ == "__main__":
    rng = np.random.default_rng(0)
    ins = {
        "x": rng.standard_normal((1, T, D), np.float32),
        "Wq": (rng.standard_normal((D, D)) * 0.02).astype(np.float32),
        "Wk": (rng.standard_normal((D, D)) * 0.02).astype(np.float32),
        "Wv": (rng.standard_normal((D, D)) * 0.02).astype(np.float32),
        "Wproj": (rng.standard_normal((D, D)) * 0.02).astype(np.float32),
        "lambda_q1": (rng.standard_normal(32) * 0.1).astype(np.float32),
        "lambda_k1": (rng.standard_normal(32) * 0.1).astype(np.float32),
        "lambda_q2": (rng.standard_normal(32) * 0.1).astype(np.float32),
        "lambda_k2": (rng.standard_normal(32) * 0.1).astype(np.float32),
    }
    y = kernel(**ins)
    print("kernel output", y.shape, y.dtype, float(np.abs(y).mean()))


# revision 29
# speedup vs baseline: 1.6904x; 1.0147x over previous
"""Trainium2 Bass kernel for MixerDiffAttention (differential attention).

Sharding: tensor-parallel over the 8 (n_head//2) head groups across 8 cores
(data-parallel over B is trivial since B=1). Each core computes the QKV
projections for its head group, both differential attention branches, the
normalized combination y1 - lambda*y2, and its head's partial product with
the row-sharded c_proj. The host sums the 8 partial outputs (the unshard
step for row-parallel tensor parallelism).

Math layout notes (per core, head h):
  - x is host-transposed to xT [D, T] so D (the contraction dim) sits on
    SBUF partitions for all projection matmuls.
  - q/k are produced in [t, c] layout (rmsnorm reduce + rotary are free-dim
    ops there), then PE-transposed to [c, t] for the score matmuls.
  - scores are computed transposed: pT[s, t] = exp(scale * q_t . k_s). Since
    q/k are RMS-normalized, |score*scale| <= 8, so exp never overflows and no
    max-subtraction is needed.
  - softmax denominator comes from an all-ones [128,128] lhsT matmul over pT,
    which broadcasts the denominator across all 128 psum partitions so the
    divide is a plain tensor_tensor multiply by a reciprocal.
  - PV matmul produces yT [j, t]; causality is handled by skipping fully
    masked s-blocks, narrowing matmuls on diagonal blocks, and masking exp
    output on the diagonal.
  - The partial projection for t-chunk tc depends only on ycomb[:, tc], so
    Tile overlaps it with the attention of later t-chunks.
"""

import os
import sys

import numpy as np

for _p in ("/opt/trn_rl_repo", "/root/.axon_site/_ro/trn_rl_repo"):
    if os.path.isdir(_p) and _p not in sys.path:
        sys.path.insert(0, _p)

import ml_dtypes

import concourse.bass as bass
import concourse.mybir as mybir
import concourse.tile as tile
from concourse import bacc
from concourse.bass import ds, ts
from concourse.bass_utils import run_bass_kernel_spmd
from concourse.masks import make_identity

BF16 = mybir.dt.bfloat16
F32 = mybir.dt.float32
AF = mybir.ActivationFunctionType
ALU = mybir.AluOpType

N_HEAD = 16
D = 1024
HD = 64  # head dim
T = 2048
NCORES = 8
TB = T // 128  # 16 t-blocks
KC = D // 128  # 8 contraction chunks
NTC = T // 512  # 4 t-chunks of 512
LAMBDA_INIT = 0.8 - 0.6 * float(np.exp(-0.3 * 1))
EPS = float(np.finfo(np.float32).eps)
SCALE = 1.0 / 8.0  # 1/sqrt(64)

_CACHE = {}


def _build_program(lam: float) -> bass.Bass:
    nc = bacc.Bacc("TRN2", target_bir_lowering=False, debug=False)

    xT = nc.declare_dram_parameter("xT", [D, T], BF16, isOutput=False)
    wqkv = nc.declare_dram_parameter("wqkv", [D, 384], BF16, isOutput=False)
    wpp = nc.declare_dram_parameter("wpp", [128, D], BF16, isOutput=False)
    cos_d = nc.declare_dram_parameter("cos", [128, TB * 32], BF16, isOutput=False)
    sin_d = nc.declare_dram_parameter("sin", [128, TB * 32], BF16, isOutput=False)
    diag_d = nc.declare_dram_parameter("diag", [128, 128], BF16, isOutput=False)
    outTp = nc.declare_dram_parameter("outTp", [D, T], F32, isOutput=True)

    with tile.TileContext(nc) as tc:
        with (
            tc.tile_pool(name="const", bufs=1) as cpool,
            tc.tile_pool(name="work", bufs=3) as wpool,
            tc.tile_pool(name="ptile", bufs=4) as ppool,
            tc.tile_pool(name="ostage", bufs=3) as opool,
            tc.tile_pool(name="pp", bufs=2, space="PSUM") as pp_pool,
            tc.tile_pool(name="py", bufs=2, space="PSUM") as py_pool,
            tc.tile_pool(name="pd", bufs=2, space="PSUM") as pd_pool,
            tc.tile_pool(name="ptr", bufs=2, space="PSUM") as ptr_pool,
        ):
            # ---- persistent SBUF tensors ----
            xT_sb = cpool.tile([128, KC, T], BF16, tag="xT")
            wqkv_sb = cpool.tile([128, KC, 384], BF16, tag="wqkv")
            wpp_sb = cpool.tile([128, KC, 128], BF16, tag="wpp")
            cos_sb = cpool.tile([128, TB, 32], BF16, tag="cos")
            sin_sb = cpool.tile([128, TB, 32], BF16, tag="sin")
            diag_sb = cpool.tile([128, 128], BF16, tag="diag")
            ones_sb = cpool.tile([128, 128], BF16, tag="ones")
            ident_sb = cpool.tile([128, 128], BF16, tag="ident")
            qT_sb = cpool.tile([128, T], BF16, tag="qT")  # rows 0:64 g0, 64:128 g1
            kT_sb = cpool.tile([128, T], BF16, tag="kT")
            v_sb = cpool.tile([128, TB, 128], BF16, tag="v")  # [s-part, tb, j]
            ycomb_sb = cpool.tile([128, T], BF16, tag="ycomb")  # [j, t]

            # ---- load constants ----
            # Critical first batch on three concurrent DMA issue paths:
            # wqkv (sync) + x t-chunk 0 (gpsimd/scalar) gate the first QKV
            # matmuls; everything else streams in behind them.
            for kc in range(KC):
                nc.sync.dma_start(out=wqkv_sb[:, kc, :], in_=wqkv[ts(kc, 128), :])
            for kc in range(KC):
                eng = nc.gpsimd if kc % 2 == 0 else nc.scalar
                eng.dma_start(
                    out=xT_sb[:, kc, ts(0, 512)], in_=xT[ts(kc, 128), ts(0, 512)]
                )
            nc.scalar.dma_start(
                out=cos_sb[:].rearrange("p a b -> p (a b)"), in_=cos_d[:, :]
            )
            nc.scalar.dma_start(
                out=sin_sb[:].rearrange("p a b -> p (a b)"), in_=sin_d[:, :]
            )
            nc.gpsimd.dma_start(out=diag_sb[:], in_=diag_d[:, :])
            for tc_i in range(1, NTC):
                for kc in range(KC):
                    eng = (nc.sync, nc.gpsimd, nc.scalar)[kc % 3]
                    eng.dma_start(
                        out=xT_sb[:, kc, ts(tc_i, 512)],
                        in_=xT[ts(kc, 128), ts(tc_i, 512)],
                    )
            for kc in range(KC):
                nc.sync.dma_start(out=wpp_sb[:, kc, :], in_=wpp[:, ts(kc, 128)])
            nc.vector.memset(ones_sb[:], 1.0)
            make_identity(nc, ident_sb[:])
            eps_sb = cpool.tile([128, 1], F32, tag="eps")
            nc.vector.memset(eps_sb[:], EPS)

            # ---- stage B: QKV projection + rmsnorm + rotary + transpose ----
            for tb in range(TB):
                pqkv = pp_pool.tile([128, 384], F32, tag="pp")
                for kc in range(KC):
                    nc.tensor.matmul(
                        pqkv[:],
                        xT_sb[:, kc, ts(tb, 128)],
                        wqkv_sb[:, kc, :],
                        start=(kc == 0),
                        stop=(kc == KC - 1),
                    )
                # v slice -> v_sb (no norm)
                nc.vector.tensor_copy(v_sb[:, tb, :], pqkv[:, 256:384])

                # sum of squares per 64-wide subhead (q1 q2 k1 k2)
                sq = wpool.tile([128, 256], F32, tag="sq")
                nc.scalar.square(sq[:], pqkv[:, 0:256])
                ssq = wpool.tile([128, 4], F32, tag="ssq")
                nc.vector.reduce_sum(
                    ssq[:],
                    sq[:].rearrange("p (h c) -> p h c", c=HD),
                    axis=mybir.AxisListType.X,
                )
                # rscale = 1/sqrt(ssq/64 + eps)
                srt = wpool.tile([128, 4], F32, tag="srt")
                nc.scalar.activation(
                    srt[:], ssq[:], AF.Sqrt, bias=eps_sb[:], scale=1.0 / HD
                )
                rsc = wpool.tile([128, 4], F32, tag="rsc")
                nc.vector.reciprocal(rsc[:], srt[:])

                normed = wpool.tile([128, 4, HD], BF16, tag="normed")
                rscb = rsc[:].unsqueeze(2).broadcast_to([128, 4, HD])
                nc.vector.tensor_mul(
                    normed[:],
                    pqkv[:, 0:256].rearrange("p (h c) -> p h c", c=HD),
                    rscb,
                )

                # rotary: out1 = n1*c + n2*s ; out2 = n2*c - n1*s
                n1 = normed[:, :, 0:32]
                n2 = normed[:, :, 32:64]
                cosb = cos_sb[:, tb, :].unsqueeze(1).broadcast_to([128, 4, 32])
                sinb = sin_sb[:, tb, :].unsqueeze(1).broadcast_to([128, 4, 32])
                rot = wpool.tile([128, 4, HD], BF16, tag="rot")
                tmp = wpool.tile([128, 4, 32], BF16, tag="rtmp")
                nc.vector.tensor_mul(tmp[:], n1, cosb)
                tmp2 = wpool.tile([128, 4, 32], BF16, tag="rtmp2")
                nc.vector.tensor_mul(tmp2[:], n2, sinb)
                nc.vector.tensor_add(rot[:, :, 0:32], tmp[:], tmp2[:])
                nc.vector.tensor_mul(tmp[:], n2, cosb)
                nc.vector.tensor_mul(tmp2[:], n1, sinb)
                nc.vector.tensor_sub(rot[:, :, 32:64], tmp[:], tmp2[:])

                # transpose q (subheads 0,1) and k (subheads 2,3) -> [c, t]
                rot2d = rot[:].rearrange("p a c -> p (a c)")
                ptq = ptr_pool.tile([128, 128], BF16, tag="ptr")
                nc.tensor.transpose(ptq[:], rot2d[:, 0:128], ident_sb[:])
                nc.scalar.copy(qT_sb[:, ts(tb, 128)], ptq[:])
                ptk = ptr_pool.tile([128, 128], BF16, tag="ptr")
                nc.tensor.transpose(ptk[:], rot2d[:, 128:256], ident_sb[:])
                nc.scalar.copy(kT_sb[:, ts(tb, 128)], ptk[:])

            # ---- stage C: differential attention + partial projection ----
            y1n_tiles = {}
            for tc_i in range(NTC):
                nsb = 4 * tc_i + 4  # s-blocks touching this t-chunk
                for g in range(2):
                    py = py_pool.tile([128, 512], F32, tag="py")
                    pdn = pd_pool.tile([128, 512], F32, tag="pd")
                    for si in range(nsb):
                        col0 = max(0, si * 128 - tc_i * 512)
                        w = 512 - col0
                        pp = pp_pool.tile([128, 512], F32, tag="pp")
                        nc.tensor.matmul(
                            pp[:, col0:512],
                            kT_sb[ds(g * 64, 64), ts(si, 128)],
                            qT_sb[ds(g * 64, 64), ds(tc_i * 512 + col0, w)],
                            start=True,
                            stop=True,
                        )
                        pt = ppool.tile([128, 512], BF16, tag="pt")
                        nc.scalar.activation(
                            pt[:, col0:512], pp[:, col0:512], AF.Exp, scale=SCALE
                        )
                        if col0 > 0 or si * 128 == tc_i * 512:
                            # diagonal block: zero out s > t inside it
                            nc.vector.tensor_mul(
                                pt[:, col0 : col0 + 128],
                                pt[:, col0 : col0 + 128],
                                diag_sb[:],
                            )
                        nc.tensor.matmul(
                            py[:, col0:512],
                            v_sb[:, si, :],
                            pt[:, col0:512],
                            start=(si == 0),
                            stop=(si == nsb - 1),
                        )
                        nc.tensor.matmul(
                            pdn[:, col0:512],
                            ones_sb[:],
                            pt[:, col0:512],
                            start=(si == 0),
                            stop=(si == nsb - 1),
                        )
                    rec = wpool.tile([128, 512], F32, tag="rec")
                    nc.vector.reciprocal_approx_fast(rec[:], pdn[:])
                    if g == 0:
                        y1n = wpool.tile([128, 512], F32, tag="y1n")
                        nc.vector.tensor_mul(y1n[:], py[:], rec[:])
                        y1n_tiles[tc_i] = y1n
                    else:
                        y2n = wpool.tile([128, 512], F32, tag="y2n")
                        nc.vector.tensor_mul(y2n[:], py[:], rec[:])
                        nc.vector.scalar_tensor_tensor(
                            ycomb_sb[:, ts(tc_i, 512)],
                            y2n[:],
                            -lam,
                            y1n_tiles[tc_i][:],
                            ALU.mult,
                            ALU.add,
                        )

                # partial projection for this t-chunk: overlaps later chunks
                for ic in range(KC):
                    po = ptr_pool.tile([128, 512], F32, tag="ptr")
                    nc.tensor.matmul(
                        po[:],
                        wpp_sb[:, ic, :],
                        ycomb_sb[:, ts(tc_i, 512)],
                        start=True,
                        stop=True,
                    )
                    ost = opool.tile([128, 512], F32, tag="ost")
                    nc.vector.tensor_copy(ost[:], po[:])
                    nc.sync.dma_start(
                        out=outTp[ts(ic, 128), ts(tc_i, 512)], in_=ost[:]
                    )

    nc.compile()
    return nc


def _make_in_maps(x, Wq, Wk, Wv, Wproj):
    bf = ml_dtypes.bfloat16
    xT = np.ascontiguousarray(x[0].T).astype(bf)  # [D, T]

    # rotary tables, rearranged to [tp, tb, 32] and flattened
    inv = 1.0 / (10000.0 ** (np.arange(0, HD, 2, dtype=np.float32) / HD))
    fr = np.outer(np.arange(T, dtype=np.float32), inv)  # [T, 32]
    cos = np.cos(fr).reshape(TB, 128, 32).transpose(1, 0, 2).reshape(128, -1)
    sin = np.sin(fr).reshape(TB, 128, 32).transpose(1, 0, 2).reshape(128, -1)
    cos, sin = cos.astype(bf), sin.astype(bf)
    diag = np.triu(np.ones((128, 128), np.float32)).astype(bf)

    in_maps = []
    for h in range(NCORES):
        wqk = np.concatenate(
            [
                Wq[h * 64 : h * 64 + 64],
                Wq[512 + h * 64 : 512 + h * 64 + 64],
                Wk[h * 64 : h * 64 + 64],
                Wk[512 + h * 64 : 512 + h * 64 + 64],
                Wv[h * 128 : h * 128 + 128],
            ],
            axis=0,
        ).T  # [D, 384]
        # wpp[j, i] = Wproj[i, h*128+j] -- lhsT chunks for the partial proj
        wpp = Wproj[:, h * 128 : (h + 1) * 128].T  # [128 j, 1024 i]
        in_maps.append(
            {
                "xT": xT,
                "wqkv": np.ascontiguousarray(wqk).astype(bf),
                "wpp": np.ascontiguousarray(wpp).astype(bf),
                "cos": cos,
                "sin": sin,
                "diag": diag,
            }
        )
    return in_maps


def _get_program(lam: float):
    key = round(lam, 10)
    if key not in _CACHE:
        _CACHE[key] = _build_program(lam)
    return _CACHE[key]


def kernel(x, Wq, Wk, Wv, Wproj, lambda_q1, lambda_k1, lambda_q2, lambda_k2):
    x = np.asarray(x, np.float32)
    Wq, Wk = np.asarray(Wq, np.float32), np.asarray(Wk, np.float32)
    Wv, Wproj = np.asarray(Wv, np.float32), np.asarray(Wproj, np.float32)

    lam1 = float(np.exp(np.sum(np.asarray(lambda_q1) * np.asarray(lambda_k1))))
    lam2 = float(np.exp(np.sum(np.asarray(lambda_q2) * np.asarray(lambda_k2))))
    lam = lam1 - lam2 + LAMBDA_INIT

    in_maps = _make_in_maps(x, Wq, Wk, Wv, Wproj)
    nc = _get_program(lam)

    res = run_bass_kernel_spmd(nc, in_maps, list(range(NCORES)))
    # unshard: row-parallel c_proj -> sum the 8 partial products
    acc = res.results[0]["outTp"].astype(np.float64)
    for h in range(1, NCORES):
        acc += res.results[h]["outTp"]
    return np.ascontiguousarray(acc.T).astype(np.float32).reshape(1, T, D)


if __name__ == "__main__":
    rng = np.random.default_rng(0)
    ins = {
        "x": rng.standard_normal((1, T, D), np.float32),
        "Wq": (rng.standard_normal((D, D)) * 0.02).astype(np.float32),
        "Wk": (rng.standard_normal((D, D)) * 0.02).astype(np.float32),
        "Wv": (rng.standard_normal((D, D)) * 0.02).astype(np.float32),
        "Wproj": (rng.standard_normal((D, D)) * 0.02).astype(np.float32),
        "lambda_q1": (rng.standard_normal(32) * 0.1).astype(np.float32),
        "lambda_k1": (rng.standard_normal(32) * 0.1).astype(np.float32),
        "lambda_q2": (rng.standard_normal(32) * 0.1).astype(np.float32),
        "lambda_k2": (rng.standard_normal(32) * 0.1).astype(np.float32),
    }
    y = kernel(**ins)
    print("kernel output", y.shape, y.dtype, float(np.abs(y).mean()))


# revision 30
# speedup vs baseline: 1.6933x; 1.0017x over previous
"""Trainium2 Bass kernel for MixerDiffAttention (differential attention).

Sharding: tensor-parallel over the 8 (n_head//2) head groups across 8 cores
(data-parallel over B is trivial since B=1). Each core computes the QKV
projections for its head group, both differential attention branches, the
normalized combination y1 - lambda*y2, and its head's partial product with
the row-sharded c_proj. The host sums the 8 partial outputs (the unshard
step for row-parallel tensor parallelism).

Math layout notes (per core, head h):
  - x is host-transposed to xT [D, T] so D (the contraction dim) sits on
    SBUF partitions for all projection matmuls.
  - q/k are produced in [t, c] layout (rmsnorm reduce + rotary are free-dim
    ops there), then PE-transposed to [c, t] for the score matmuls.
  - scores are computed transposed: pT[s, t] = exp(scale * q_t . k_s). Since
    q/k are RMS-normalized, |score*scale| <= 8, so exp never overflows and no
    max-subtraction is needed.
  - softmax denominator comes from an all-ones [128,128] lhsT matmul over pT,
    which broadcasts the denominator across all 128 psum partitions so the
    divide is a plain tensor_tensor multiply by a reciprocal.
  - PV matmul produces yT [j, t]; causality is handled by skipping fully
    masked s-blocks, narrowing matmuls on diagonal blocks, and masking exp
    output on the diagonal.
  - The partial projection for t-chunk tc depends only on ycomb[:, tc], so
    Tile overlaps it with the attention of later t-chunks.
"""

import os
import sys

import numpy as np

for _p in ("/opt/trn_rl_repo", "/root/.axon_site/_ro/trn_rl_repo"):
    if os.path.isdir(_p) and _p not in sys.path:
        sys.path.insert(0, _p)

import ml_dtypes

import concourse.bass as bass
import concourse.mybir as mybir
import concourse.tile as tile
from concourse import bacc
from concourse.bass import ds, ts
from concourse.bass_utils import run_bass_kernel_spmd
from concourse.masks import make_identity

BF16 = mybir.dt.bfloat16
F32 = mybir.dt.float32
AF = mybir.ActivationFunctionType
ALU = mybir.AluOpType

N_HEAD = 16
D = 1024
HD = 64  # head dim
T = 2048
NCORES = 8
TB = T // 128  # 16 t-blocks
KC = D // 128  # 8 contraction chunks
NTC = T // 512  # 4 t-chunks of 512
LAMBDA_INIT = 0.8 - 0.6 * float(np.exp(-0.3 * 1))
EPS = float(np.finfo(np.float32).eps)
SCALE = 1.0 / 8.0  # 1/sqrt(64)

_CACHE = {}


def _build_program(lam: float) -> bass.Bass:
    nc = bacc.Bacc("TRN2", target_bir_lowering=False, debug=False)

    xT = nc.declare_dram_parameter("xT", [D, T], BF16, isOutput=False)
    wqkv = nc.declare_dram_parameter("wqkv", [D, 384], BF16, isOutput=False)
    wpp = nc.declare_dram_parameter("wpp", [128, D], BF16, isOutput=False)
    cos_d = nc.declare_dram_parameter("cos", [128, TB * 32], BF16, isOutput=False)
    sin_d = nc.declare_dram_parameter("sin", [128, TB * 32], BF16, isOutput=False)
    diag_d = nc.declare_dram_parameter("diag", [128, 128], BF16, isOutput=False)
    outTp = nc.declare_dram_parameter("outTp", [D, T], F32, isOutput=True)

    with tile.TileContext(nc) as tc:
        with (
            tc.tile_pool(name="const", bufs=1) as cpool,
            tc.tile_pool(name="work", bufs=4) as wpool,
            tc.tile_pool(name="ptile", bufs=6) as ppool,
            tc.tile_pool(name="ostage", bufs=3) as opool,
            tc.tile_pool(name="pp", bufs=2, space="PSUM") as pp_pool,
            tc.tile_pool(name="py", bufs=2, space="PSUM") as py_pool,
            tc.tile_pool(name="pd", bufs=2, space="PSUM") as pd_pool,
            tc.tile_pool(name="ptr", bufs=2, space="PSUM") as ptr_pool,
        ):
            # ---- persistent SBUF tensors ----
            xT_sb = cpool.tile([128, KC, T], BF16, tag="xT")
            wqkv_sb = cpool.tile([128, KC, 384], BF16, tag="wqkv")
            wpp_sb = cpool.tile([128, KC, 128], BF16, tag="wpp")
            cos_sb = cpool.tile([128, TB, 32], BF16, tag="cos")
            sin_sb = cpool.tile([128, TB, 32], BF16, tag="sin")
            diag_sb = cpool.tile([128, 128], BF16, tag="diag")
            ones_sb = cpool.tile([128, 128], BF16, tag="ones")
            ident_sb = cpool.tile([128, 128], BF16, tag="ident")
            qT_sb = cpool.tile([128, T], BF16, tag="qT")  # rows 0:64 g0, 64:128 g1
            kT_sb = cpool.tile([128, T], BF16, tag="kT")
            v_sb = cpool.tile([128, TB, 128], BF16, tag="v")  # [s-part, tb, j]
            ycomb_sb = cpool.tile([128, T], BF16, tag="ycomb")  # [j, t]

            # ---- load constants ----
            # Critical first batch on three concurrent DMA issue paths:
            # wqkv (sync) + x t-chunk 0 (gpsimd/scalar) gate the first QKV
            # matmuls; everything else streams in behind them.
            for kc in range(KC):
                nc.sync.dma_start(out=wqkv_sb[:, kc, :], in_=wqkv[ts(kc, 128), :])
            for kc in range(KC):
                eng = nc.gpsimd if kc % 2 == 0 else nc.scalar
                eng.dma_start(
                    out=xT_sb[:, kc, ts(0, 512)], in_=xT[ts(kc, 128), ts(0, 512)]
                )
            nc.scalar.dma_start(
                out=cos_sb[:].rearrange("p a b -> p (a b)"), in_=cos_d[:, :]
            )
            nc.scalar.dma_start(
                out=sin_sb[:].rearrange("p a b -> p (a b)"), in_=sin_d[:, :]
            )
            nc.gpsimd.dma_start(out=diag_sb[:], in_=diag_d[:, :])
            for tc_i in range(1, NTC):
                for kc in range(KC):
                    eng = (nc.sync, nc.gpsimd, nc.scalar)[kc % 3]
                    eng.dma_start(
                        out=xT_sb[:, kc, ts(tc_i, 512)],
                        in_=xT[ts(kc, 128), ts(tc_i, 512)],
                    )
            for kc in range(KC):
                nc.sync.dma_start(out=wpp_sb[:, kc, :], in_=wpp[:, ts(kc, 128)])
            nc.vector.memset(ones_sb[:], 1.0)
            make_identity(nc, ident_sb[:])
            eps_sb = cpool.tile([128, 1], F32, tag="eps")
            nc.vector.memset(eps_sb[:], EPS)

            # ---- stage B: QKV projection + rmsnorm + rotary + transpose ----
            for tb in range(TB):
                pqkv = pp_pool.tile([128, 384], F32, tag="pp")
                for kc in range(KC):
                    nc.tensor.matmul(
                        pqkv[:],
                        xT_sb[:, kc, ts(tb, 128)],
                        wqkv_sb[:, kc, :],
                        start=(kc == 0),
                        stop=(kc == KC - 1),
                    )
                # v slice -> v_sb (no norm)
                nc.vector.tensor_copy(v_sb[:, tb, :], pqkv[:, 256:384])

                # sum of squares per 64-wide subhead (q1 q2 k1 k2)
                sq = wpool.tile([128, 256], F32, tag="sq")
                nc.scalar.square(sq[:], pqkv[:, 0:256])
                ssq = wpool.tile([128, 4], F32, tag="ssq")
                nc.vector.reduce_sum(
                    ssq[:],
                    sq[:].rearrange("p (h c) -> p h c", c=HD),
                    axis=mybir.AxisListType.X,
                )
                # rscale = 1/sqrt(ssq/64 + eps)
                srt = wpool.tile([128, 4], F32, tag="srt")
                nc.scalar.activation(
                    srt[:], ssq[:], AF.Sqrt, bias=eps_sb[:], scale=1.0 / HD
                )
                rsc = wpool.tile([128, 4], F32, tag="rsc")
                nc.vector.reciprocal(rsc[:], srt[:])

                normed = wpool.tile([128, 4, HD], BF16, tag="normed")
                rscb = rsc[:].unsqueeze(2).broadcast_to([128, 4, HD])
                nc.vector.tensor_mul(
                    normed[:],
                    pqkv[:, 0:256].rearrange("p (h c) -> p h c", c=HD),
                    rscb,
                )

                # rotary: out1 = n1*c + n2*s ; out2 = n2*c - n1*s
                n1 = normed[:, :, 0:32]
                n2 = normed[:, :, 32:64]
                cosb = cos_sb[:, tb, :].unsqueeze(1).broadcast_to([128, 4, 32])
                sinb = sin_sb[:, tb, :].unsqueeze(1).broadcast_to([128, 4, 32])
                rot = wpool.tile([128, 4, HD], BF16, tag="rot")
                tmp = wpool.tile([128, 4, 32], BF16, tag="rtmp")
                nc.vector.tensor_mul(tmp[:], n1, cosb)
                tmp2 = wpool.tile([128, 4, 32], BF16, tag="rtmp2")
                nc.vector.tensor_mul(tmp2[:], n2, sinb)
                nc.vector.tensor_add(rot[:, :, 0:32], tmp[:], tmp2[:])
                nc.vector.tensor_mul(tmp[:], n2, cosb)
                nc.vector.tensor_mul(tmp2[:], n1, sinb)
                nc.vector.tensor_sub(rot[:, :, 32:64], tmp[:], tmp2[:])

                # transpose q (subheads 0,1) and k (subheads 2,3) -> [c, t]
                rot2d = rot[:].rearrange("p a c -> p (a c)")
                ptq = ptr_pool.tile([128, 128], BF16, tag="ptr")
                nc.tensor.transpose(ptq[:], rot2d[:, 0:128], ident_sb[:])
                nc.scalar.copy(qT_sb[:, ts(tb, 128)], ptq[:])
                ptk = ptr_pool.tile([128, 128], BF16, tag="ptr")
                nc.tensor.transpose(ptk[:], rot2d[:, 128:256], ident_sb[:])
                nc.scalar.copy(kT_sb[:, ts(tb, 128)], ptk[:])

            # ---- stage C: differential attention + partial projection ----
            y1n_tiles = {}
            for tc_i in range(NTC):
                nsb = 4 * tc_i + 4  # s-blocks touching this t-chunk
                for g in range(2):
                    py = py_pool.tile([128, 512], F32, tag="py")
                    pdn = pd_pool.tile([128, 512], F32, tag="pd")
                    for si in range(nsb):
                        col0 = max(0, si * 128 - tc_i * 512)
                        w = 512 - col0
                        pp = pp_pool.tile([128, 512], F32, tag="pp")
                        nc.tensor.matmul(
                            pp[:, col0:512],
                            kT_sb[ds(g * 64, 64), ts(si, 128)],
                            qT_sb[ds(g * 64, 64), ds(tc_i * 512 + col0, w)],
                            start=True,
                            stop=True,
                        )
                        pt = ppool.tile([128, 512], BF16, tag="pt")
                        nc.scalar.activation(
                            pt[:, col0:512], pp[:, col0:512], AF.Exp, scale=SCALE
                        )
                        if col0 > 0 or si * 128 == tc_i * 512:
                            # diagonal block: zero out s > t inside it
                            nc.vector.tensor_mul(
                                pt[:, col0 : col0 + 128],
                                pt[:, col0 : col0 + 128],
                                diag_sb[:],
                            )
                        nc.tensor.matmul(
                            py[:, col0:512],
                            v_sb[:, si, :],
                            pt[:, col0:512],
                            start=(si == 0),
                            stop=(si == nsb - 1),
                        )
                        nc.tensor.matmul(
                            pdn[:, col0:512],
                            ones_sb[:],
                            pt[:, col0:512],
                            start=(si == 0),
                            stop=(si == nsb - 1),
                        )
                    rec = wpool.tile([128, 512], F32, tag="rec")
                    nc.vector.reciprocal_approx_fast(rec[:], pdn[:])
                    if g == 0:
                        y1n = wpool.tile([128, 512], F32, tag="y1n")
                        nc.vector.tensor_mul(y1n[:], py[:], rec[:])
                        y1n_tiles[tc_i] = y1n
                    else:
                        y2n = wpool.tile([128, 512], F32, tag="y2n")
                        nc.vector.tensor_mul(y2n[:], py[:], rec[:])
                        nc.vector.scalar_tensor_tensor(
                            ycomb_sb[:, ts(tc_i, 512)],
                            y2n[:],
                            -lam,
                            y1n_tiles[tc_i][:],
                            ALU.mult,
                            ALU.add,
                        )

                # partial projection for this t-chunk: overlaps later chunks
                for ic in range(KC):
                    po = ptr_pool.tile([128, 512], F32, tag="ptr")
                    nc.tensor.matmul(
                        po[:],
                        wpp_sb[:, ic, :],
                        ycomb_sb[:, ts(tc_i, 512)],
                        start=True,
                        stop=True,
                    )
                    ost = opool.tile([128, 512], F32, tag="ost")
                    nc.vector.tensor_copy(ost[:], po[:])
                    nc.sync.dma_start(
                        out=outTp[ts(ic, 128), ts(tc_i, 512)], in_=ost[:]
                    )

    nc.compile()
    return nc


def _make_in_maps(x, Wq, Wk, Wv, Wproj):
    bf = ml_dtypes.bfloat16
    xT = np.ascontiguousarray(x[0].T).astype(bf)  # [D, T]

    # rotary tables, rearranged to [tp, tb, 32] and flattened
    inv = 1.0 / (10000.0 ** (np.arange(0, HD, 2, dtype=np.float32) / HD))
    fr = np.outer(np.arange(T, dtype=np.float32), inv)  # [T, 32]
    cos = np.cos(fr).reshape(TB, 128, 32).transpose(1, 0, 2).reshape(128, -1)
    sin = np.sin(fr).reshape(TB, 128, 32).transpose(1, 0, 2).reshape(128, -1)
    cos, sin = cos.astype(bf), sin.astype(bf)
    diag = np.triu(np.ones((128, 128), np.float32)).astype(bf)

    in_maps = []
    for h in range(NCORES):
        wqk = np.concatenate(
            [
                Wq[h * 64 : h * 64 + 64],
                Wq[512 + h * 64 : 512 + h * 64 + 64],
                Wk[h * 64 : h * 64 + 64],
                Wk[512 + h * 64 : 512 + h * 64 + 64],
                Wv[h * 128 : h * 128 + 128],
            ],
            axis=0,
        ).T  # [D, 384]
        # wpp[j, i] = Wproj[i, h*128+j] -- lhsT chunks for the partial proj
        wpp = Wproj[:, h * 128 : (h + 1) * 128].T  # [128 j, 1024 i]
        in_maps.append(
            {
                "xT": xT,
                "wqkv": np.ascontiguousarray(wqk).astype(bf),
                "wpp": np.ascontiguousarray(wpp).astype(bf),
                "cos": cos,
                "sin": sin,
                "diag": diag,
            }
        )
    return in_maps


def _get_program(lam: float):
    key = round(lam, 10)
    if key not in _CACHE:
        _CACHE[key] = _build_program(lam)
    return _CACHE[key]


def kernel(x, Wq, Wk, Wv, Wproj, lambda_q1, lambda_k1, lambda_q2, lambda_k2):
    x = np.asarray(x, np.float32)
    Wq, Wk = np.asarray(Wq, np.float32), np.asarray(Wk, np.float32)
    Wv, Wproj = np.asarray(Wv, np.float32), np.asarray(Wproj, np.float32)

    lam1 = float(np.exp(np.sum(np.asarray(lambda_q1) * np.asarray(lambda_k1))))
    lam2 = float(np.exp(np.sum(np.asarray(lambda_q2) * np.asarray(lambda_k2))))
    lam = lam1 - lam2 + LAMBDA_INIT

    in_maps = _make_in_maps(x, Wq, Wk, Wv, Wproj)
    nc = _get_program(lam)

    res = run_bass_kernel_spmd(nc, in_maps, list(range(NCORES)))
    # unshard: row-parallel c_proj -> sum the 8 partial products
    acc = res.results[0]["outTp"].astype(np.float64)
    for h in range(1, NCORES):
        acc += res.results[h]["outTp"]
    return np.ascontiguousarray(acc.T).astype(np.float32).reshape(1, T, D)


if __name__ == "__main__":
    rng = np.random.default_rng(0)
    ins = {
        "x": rng.standard_normal((1, T, D), np.float32),
        "Wq": (rng.standard_normal((D, D)) * 0.02).astype(np.float32),
        "Wk": (rng.standard_normal((D, D)) * 0.02).astype(np.float32),
        "Wv": (rng.standard_normal((D, D)) * 0.02).astype(np.float32),
        "Wproj": (rng.standard_normal((D, D)) * 0.02).astype(np.float32),
        "lambda_q1": (rng.standard_normal(32) * 0.1).astype(np.float32),
        "lambda_k1": (rng.standard_normal(32) * 0.1).astype(np.float32),
        "lambda_q2": (rng.standard_normal(32) * 0.1).astype(np.float32),
        "lambda_k2": (rng.standard_normal(32) * 0.1).astype(np.float32),
    }
    y = kernel(**ins)
    print("kernel output", y.shape, y.dtype, float(np.abs(y).mean()))
